# revision 9
# baseline (speedup 1.0000x reference)
"""LoRA-with-routing kernel for Trainium2 (8 NeuronCores, SPMD).

out[b] = base[b] + (x[b] @ lora_A[idx[b]]) @ lora_B[idx[b]] * s[idx[b]]

Sharding: data-parallel over batch (B=8 rows, one per core). The adapter
gather (routing) happens host-side while sharding: each core receives its
batch row plus that row's adapter weights (scale folded into B).

The kernel is HBM-bandwidth-bound; streams use the narrowest dtype the
2e-2 relative-error budget allows:
  x    -> fp8 e4m3  (8 MiB;  GEMM1 rhs)
  A    -> fp8 e4m3, pre-scaled x256 so values are e4m3-normal
          (the 1/256 is folded into B host-side)
  base -> bf16      (16 MiB)
  out  -> bf16      (16 MiB; host upcasts to f32 after gather)

With both GEMM1 operands in e4m3 the PE runs DoubleRow perf mode
(contraction 256 per pass, ~1.6x on GEMM1's column stream).

The base+y add runs INSIDE the base-load DMA (SWDGE CCE accum_op=add),
so the only elementwise work is the PSUM evacuation of y, split DVE/ACT.
Per 512-token group g:
  1. 2 MiB DMA loads xg[p, c, t] (Pool queue, all queued up front)
  2. GEMM1 (PE, DoubleRow): it_ps[64 r, 512 t] += A_{2c}.T @ xg_{2c}
  3. DVE evac it_ps -> bf16
  4. per 128-token tile: GEMM2 y[128,512] = it.T @ B per o-chunk; evac
     y -> ob bf16 (DVE even o / ACT odd o); CCE-accum base into ob
     (Pool); store ob (Sync)
Queues: Pool = a+x then base-accum loads, Sync = stores, ACT = b + tail
stores. No DMA queue ever waits on compute that isn't its own payload's
dependency.
"""

import sys

for _p in ("/opt/trn_rl_repo", "/root/.axon_site/_ro/trn_rl_repo"):
    if _p not in sys.path:
        sys.path.append(_p)

import numpy as np
import ml_dtypes

import concourse.bass as bass
import concourse.bacc as bacc
import concourse.mybir as mybir
from concourse import tile

B, T, D, R = 8, 2048, 4096, 64
P = 128          # partitions
DC = D // P      # 32 d-chunks (GEMM1 contraction)
TG = 512         # token group (GEMM1 moving dim, one PSUM bank of f32)
NG = T // TG     # 4 token groups
NT = T // P      # 16 token tiles of 128
OCH = 512        # output free chunk (one PSUM bank of f32)
OC = D // OCH    # 8 o-chunks
A_SCALE = 256.0  # host A prescale so e4m3 values are normal

F32 = mybir.dt.float32
BF16 = mybir.dt.bfloat16
FP8 = mybir.dt.float8e4   # e4m3: matches ml_dtypes.float8_e4m3 (max 240)


def build_program(t_tokens: int = T):
    ng = t_tokens // TG
    nc = bacc.Bacc("TRN2", target_bir_lowering=False, debug=False, num_devices=B)
    xt = nc.dram_tensor("xt", [ng * P, DC * TG], FP8, kind="ExternalInput").ap()
    base = nc.dram_tensor("base", [t_tokens, D], BF16, kind="ExternalInput").ap()
    a_w = nc.dram_tensor("a_w", [P, DC * R], FP8, kind="ExternalInput").ap()
    b_w = nc.dram_tensor("b_w", [R, D], BF16, kind="ExternalInput").ap()
    out = nc.dram_tensor("out", [t_tokens, D], BF16, kind="ExternalOutput").ap()

    with tile.TileContext(nc) as tc:
        _body(tc, xt, base, a_w, b_w, out, ng)
    nc.compile()
    return nc


def _body(tc, xt, base, a_w, b_w, out, ng):
    nc = tc.nc
    nt = ng * (TG // P)
    with (
        tc.tile_pool(name="const", bufs=1) as cpool,
        tc.tile_pool(name="xg", bufs=4) as x_pool,
        tc.tile_pool(name="ob", bufs=6) as ob_pool,
        tc.tile_pool(name="it", bufs=2) as it_pool,
        tc.tile_pool(name="ps1", bufs=2, space="PSUM") as ps1,
        tc.tile_pool(name="ps2", bufs=6, space="PSUM") as ps2,
    ):
        # Adapter weights, loaded once. a_sb[p, c, r] = A[c*128+p, r] * 256.
        a_sb = cpool.tile([P, DC, R], FP8)
        nc.gpsimd.dma_start(a_sb[:], a_w[:].rearrange("p (c r) -> p c r", r=R))
        b_sb = cpool.tile([R, D], BF16)
        nc.scalar.dma_start(b_sb[:], b_w[:])

        # All x DMAs queued up front on the Pool ring so nothing ever
        # queues ahead of the GEMM1-critical stream.
        xgs = []
        for g in range(ng):
            xg = x_pool.tile([P, DC, TG], FP8, name="xg")
            nc.gpsimd.dma_start(
                xg[:], xt[g * P : (g + 1) * P, :].rearrange("p (c t) -> p c t", t=TG)
            )
            xgs.append(xg)

        for g in range(ng):
            # GEMM1: it_ps[r, t] = sum_c A_c.T @ xg_c, accumulated in PSUM.
            # DoubleRow: two 128-deep d-chunks contract per pass.
            xg = xgs[g]
            it_ps = ps1.tile([R, TG], F32)
            for c2 in range(DC // 2):
                nc.tensor.matmul(
                    it_ps[:],
                    a_sb[:, 2 * c2 : 2 * c2 + 2, :],
                    xg[:, 2 * c2 : 2 * c2 + 2, :],
                    start=(c2 == 0),
                    stop=(c2 == DC // 2 - 1),
                    perf_mode=mybir.MatmulPerfMode.DoubleRow,
                )
            it_sb = it_pool.tile([R, TG], BF16)
            nc.vector.tensor_copy(it_sb[:], it_ps[:])

            for sub in range(TG // P):
                k = g * (TG // P) + sub
                ob = ob_pool.tile([P, D], BF16)
                last_tile = k == nt - 1
                for o in range(OC):
                    y_ps = ps2.tile([P, OCH], F32)
                    nc.tensor.matmul(
                        y_ps[:],
                        it_sb[:, sub * P : (sub + 1) * P],
                        b_sb[:, o * OCH : (o + 1) * OCH],
                        start=True,
                        stop=True,
                    )
                    # evacuate y to bf16; the base add happens in the DMA
                    dst = ob[:, o * OCH : (o + 1) * OCH]
                    if o % 2 == 0:
                        nc.vector.tensor_copy(dst, y_ps[:])
                    else:
                        nc.scalar.activation(
                            dst, y_ps[:], mybir.ActivationFunctionType.Copy
                        )
                # ob += base, computed by the SDMA CCE during the load.
                # CCE descriptors top out at 2048 elements, so split the
                # row into 1024-element chunks.
                for q in range(4):
                    nc.gpsimd.dma_start(
                        ob[:, q * D // 4 : (q + 1) * D // 4],
                        base[k * P : (k + 1) * P, q * D // 4 : (q + 1) * D // 4],
                        accum_op=mybir.AluOpType.add,
                    )
                if last_tile:
                    # drain the tail in quarter-row stores on two queues
                    for q in range(4):
                        eng = nc.sync if q % 2 == 0 else nc.scalar
                        eng.dma_start(
                            out[k * P : (k + 1) * P, q * D // 4 : (q + 1) * D // 4],
                            ob[:, q * D // 4 : (q + 1) * D // 4],
                        )
                else:
                    nc.sync.dma_start(out[k * P : (k + 1) * P, :], ob[:])


def shard_inputs(x, base_output, adapter_indices, lora_A, lora_B, lora_scaling):
    idx = np.asarray(adapter_indices).astype(np.int64)
    a_b = np.asarray(lora_A, dtype=np.float32)[idx]        # [B, D, R]
    b_b = np.asarray(lora_B, dtype=np.float32)[idx]        # [B, R, D]
    s_b = np.asarray(lora_scaling, dtype=np.float32)[idx]  # [B]
    b_scaled = (b_b * (s_b[:, None, None] / A_SCALE)).astype(ml_dtypes.bfloat16)
    xs = np.asarray(x, dtype=np.float32)
    bs = np.asarray(base_output, dtype=np.float32)
    maps = []
    for b in range(B):
        # xt[g*P + p, c*TG + t] = x[g*TG + t, c*P + p]
        x8 = xs[b].astype(ml_dtypes.float8_e4m3)           # [T, D]
        xt = x8.reshape(NG, TG, DC, P).transpose(0, 3, 2, 1).reshape(NG * P, DC * TG)
        # a_w[p, c*R + r] = A[c*P + p, r] * 256
        a8 = (a_b[b] * A_SCALE).astype(ml_dtypes.float8_e4m3)
        a_w = a8.reshape(DC, P, R).transpose(1, 0, 2).reshape(P, DC * R)
        maps.append(
            {
                "xt": np.ascontiguousarray(xt),
                "base": bs[b].astype(ml_dtypes.bfloat16),
                "a_w": np.ascontiguousarray(a_w),
                "b_w": np.ascontiguousarray(b_scaled[b]),
            }
        )
    return maps


def run(inputs: dict, trace: bool = False, **kwargs):
    """Build + run on 8 cores. Returns (output [B,T,D] f32, BassKernelResults)."""
    from concourse.bass_utils import run_bass_kernel_spmd

    nc = build_program()
    in_maps = shard_inputs(**inputs)
    res = run_bass_kernel_spmd(
        nc, in_maps, core_ids=list(range(B)), trace=trace, **kwargs
    )
    out = np.stack([res.results[b]["out"] for b in range(B)], axis=0).astype(np.float32)
    return out, res


def kernel(x, base_output, adapter_indices, lora_A, lora_B, lora_scaling):
    out, _ = run(
        dict(
            x=x,
            base_output=base_output,
            adapter_indices=adapter_indices,
            lora_A=lora_A,
            lora_B=lora_B,
            lora_scaling=lora_scaling,
        )
    )
    return out


# revision 10
# speedup vs baseline: 1.4586x; 1.4586x over previous
"""LoRA-with-routing kernel for Trainium2 (8 NeuronCores, SPMD).

out[b] = base[b] + (x[b] @ lora_A[idx[b]]) @ lora_B[idx[b]] * s[idx[b]]

Sharding: data-parallel over batch (B=8 rows, one per core). The adapter
gather (routing) happens host-side while sharding: each core receives its
batch row plus that row's adapter weights (scales folded into B).

The kernel is HBM-bandwidth-bound; streams use the narrowest dtype the
2e-2 relative-error budget allows (~25 MiB/core total):
  x    -> fp8 e4m3 (8 MiB), A -> fp8 e4m3 prescaled x256 (values normal)
  base -> int8 (8 MiB), quantized host-side with per-core scale q
  out  -> int8 (8 MiB), same scale; host dequantizes to f32
B absorbs s/(256*q) so GEMM2's PSUM result is y/q directly, and the
device add is just base_i8 + y/q -> round to int8.

With both GEMM1 operands in e4m3 the PE runs DoubleRow perf mode
(contraction 256 per pass, ~1.6x on GEMM1's column stream).

Per 512-token group g:
  1. 2 MiB DMA loads xg[p, c, t] (Pool queue, all queued up front)
  2. GEMM1 (PE, DoubleRow): it_ps[64 r, 512 t] += A_{2c}.T @ xg_{2c}
  3. DVE evac it_ps -> bf16
  4. per 128-token tile: GEMM2 y/q [128,512] per o-chunk; DVE adds
     base_i8 + y/q from PSUM straight into the int8 out tile; store row
Queues stay DMA-pure: Pool = a+x, ACT = b+base ring, Sync = stores.
"""

import sys

for _p in ("/opt/trn_rl_repo", "/root/.axon_site/_ro/trn_rl_repo"):
    if _p not in sys.path:
        sys.path.append(_p)

import numpy as np
import ml_dtypes

import concourse.bass as bass
import concourse.bacc as bacc
import concourse.mybir as mybir
from concourse import tile

B, T, D, R = 8, 2048, 4096, 64
P = 128          # partitions
DC = D // P      # 32 d-chunks (GEMM1 contraction)
TG = 512         # token group (GEMM1 moving dim, one PSUM bank of f32)
NG = T // TG     # 4 token groups
NT = T // P      # 16 token tiles of 128
OCH = 512        # output free chunk (one PSUM bank of f32)
OC = D // OCH    # 8 o-chunks
PF = 8           # base-load prefetch depth (bs_pool bufs)
A_SCALE = 256.0  # host A prescale so e4m3 values are normal
Y_PAD = 1.25     # int8 range headroom for y on top of max|base|

F32 = mybir.dt.float32
BF16 = mybir.dt.bfloat16
FP8 = mybir.dt.float8e4   # e4m3: matches ml_dtypes.float8_e4m3 (max 240)
I8 = mybir.dt.int8


def build_program(t_tokens: int = T):
    ng = t_tokens // TG
    nc = bacc.Bacc("TRN2", target_bir_lowering=False, debug=False, num_devices=B)
    xt = nc.dram_tensor("xt", [ng * P, DC * TG], FP8, kind="ExternalInput").ap()
    base = nc.dram_tensor("base", [t_tokens, D], I8, kind="ExternalInput").ap()
    a_w = nc.dram_tensor("a_w", [P, DC * R], FP8, kind="ExternalInput").ap()
    b_w = nc.dram_tensor("b_w", [R, D], BF16, kind="ExternalInput").ap()
    out = nc.dram_tensor("out", [t_tokens, D], I8, kind="ExternalOutput").ap()

    with tile.TileContext(nc) as tc:
        _body(tc, xt, base, a_w, b_w, out, ng)
    nc.compile()
    return nc


def _body(tc, xt, base, a_w, b_w, out, ng):
    nc = tc.nc
    nt = ng * (TG // P)
    with (
        tc.tile_pool(name="const", bufs=1) as cpool,
        tc.tile_pool(name="xg", bufs=4) as x_pool,
        tc.tile_pool(name="bs", bufs=PF) as bs_pool,
        tc.tile_pool(name="ob", bufs=6) as ob_pool,
        tc.tile_pool(name="it", bufs=2) as it_pool,
        tc.tile_pool(name="ps1", bufs=2, space="PSUM") as ps1,
        tc.tile_pool(name="ps2", bufs=6, space="PSUM") as ps2,
    ):
        # Adapter weights, loaded once. a_sb[p, c, r] = A[c*128+p, r] * 256.
        a_sb = cpool.tile([P, DC, R], FP8)
        nc.gpsimd.dma_start(a_sb[:], a_w[:].rearrange("p (c r) -> p c r", r=R))
        b_sb = cpool.tile([R, D], BF16)
        nc.scalar.dma_start(b_sb[:], b_w[:])

        # All x DMAs queued up front on the Pool ring so nothing ever
        # queues ahead of the GEMM1-critical stream.
        xgs = []
        for g in range(ng):
            xg = x_pool.tile([P, DC, TG], FP8, name="xg")
            nc.gpsimd.dma_start(
                xg[:], xt[g * P : (g + 1) * P, :].rearrange("p (c t) -> p c t", t=TG)
            )
            xgs.append(xg)

        # Base-row prefetch ring on the ACT queue (DMA-pure).
        bs_tiles = {}

        def load_base(k):
            bs_tiles[k] = bs_pool.tile([P, D], I8, name="bs")
            nc.scalar.dma_start(bs_tiles[k][:], base[k * P : (k + 1) * P, :])

        for k in range(min(PF, nt)):
            load_base(k)

        for g in range(ng):
            # GEMM1: it_ps[r, t] = sum_c A_c.T @ xg_c, accumulated in PSUM.
            # DoubleRow: two 128-deep d-chunks contract per pass.
            xg = xgs[g]
            it_ps = ps1.tile([R, TG], F32)
            for c2 in range(DC // 2):
                nc.tensor.matmul(
                    it_ps[:],
                    a_sb[:, 2 * c2 : 2 * c2 + 2, :],
                    xg[:, 2 * c2 : 2 * c2 + 2, :],
                    start=(c2 == 0),
                    stop=(c2 == DC // 2 - 1),
                    perf_mode=mybir.MatmulPerfMode.DoubleRow,
                )
            it_sb = it_pool.tile([R, TG], BF16)
            nc.vector.tensor_copy(it_sb[:], it_ps[:])

            for sub in range(TG // P):
                k = g * (TG // P) + sub
                bs = bs_tiles.pop(k)
                ob = ob_pool.tile([P, D], I8)
                last_tile = k == nt - 1
                for o in range(OC):
                    y_ps = ps2.tile([P, OCH], F32)
                    nc.tensor.matmul(
                        y_ps[:],
                        it_sb[:, sub * P : (sub + 1) * P],
                        b_sb[:, o * OCH : (o + 1) * OCH],
                        start=True,
                        stop=True,
                    )
                    # out_i8 = base_i8 + y/q, rounded on the int8 write.
                    # DVE only: gpsimd has no PSUM access, ACT no 2-tensor op.
                    nc.vector.tensor_add(
                        ob[:, o * OCH : (o + 1) * OCH],
                        bs[:, o * OCH : (o + 1) * OCH],
                        y_ps[:],
                    )
                if last_tile:
                    # drain the tail in quarter-row stores on two queues
                    for q in range(4):
                        eng = nc.sync if q % 2 == 0 else nc.scalar
                        eng.dma_start(
                            out[k * P : (k + 1) * P, q * D // 4 : (q + 1) * D // 4],
                            ob[:, q * D // 4 : (q + 1) * D // 4],
                        )
                else:
                    nc.sync.dma_start(out[k * P : (k + 1) * P, :], ob[:])
                if k + PF < nt:
                    load_base(k + PF)


def shard_inputs(x, base_output, adapter_indices, lora_A, lora_B, lora_scaling):
    idx = np.asarray(adapter_indices).astype(np.int64)
    a_b = np.asarray(lora_A, dtype=np.float32)[idx]        # [B, D, R]
    b_b = np.asarray(lora_B, dtype=np.float32)[idx]        # [B, R, D]
    s_b = np.asarray(lora_scaling, dtype=np.float32)[idx]  # [B]
    xs = np.asarray(x, dtype=np.float32)
    bs = np.asarray(base_output, dtype=np.float32)
    maps = []
    qs = []
    for b in range(B):
        # xt[g*P + p, c*TG + t] = x[g*TG + t, c*P + p]
        x8 = xs[b].astype(ml_dtypes.float8_e4m3)           # [T, D]
        xt = x8.reshape(NG, TG, DC, P).transpose(0, 3, 2, 1).reshape(NG * P, DC * TG)
        # a_w[p, c*R + r] = A[c*P + p, r] * 256
        a8 = (a_b[b] * A_SCALE).astype(ml_dtypes.float8_e4m3)
        a_w = a8.reshape(DC, P, R).transpose(1, 0, 2).reshape(P, DC * R)
        # int8 quantization: shared scale for base and out
        q = float(np.abs(bs[b]).max() + Y_PAD) / 127.0
        qs.append(q)
        base_i8 = np.clip(np.rint(bs[b] / q), -127, 127).astype(np.int8)
        b_scaled = (b_b[b] * (s_b[b] / (A_SCALE * q))).astype(ml_dtypes.bfloat16)
        maps.append(
            {
                "xt": np.ascontiguousarray(xt),
                "base": base_i8,
                "a_w": np.ascontiguousarray(a_w),
                "b_w": np.ascontiguousarray(b_scaled),
            }
        )
    return maps, qs


def run(inputs: dict, trace: bool = False, **kwargs):
    """Build + run on 8 cores. Returns (output [B,T,D] f32, BassKernelResults)."""
    from concourse.bass_utils import run_bass_kernel_spmd

    nc = build_program()
    in_maps, qs = shard_inputs(**inputs)
    res = run_bass_kernel_spmd(
        nc, in_maps, core_ids=list(range(B)), trace=trace, **kwargs
    )
    out = np.stack(
        [res.results[b]["out"].astype(np.float32) * qs[b] for b in range(B)], axis=0
    )
    return out, res


def kernel(x, base_output, adapter_indices, lora_A, lora_B, lora_scaling):
    out, _ = run(
        dict(
            x=x,
            base_output=base_output,
            adapter_indices=adapter_indices,
            lora_A=lora_A,
            lora_B=lora_B,
            lora_scaling=lora_scaling,
        )
    )
    return out
